# revision 29
# baseline (speedup 1.0000x reference)
"""Trainium2 Bass kernel for nn_MixChan (dense_mlp).

Reference computation (per batch sample b):
    d   = dist / dist.sum()                       # (32,)
    xs  = x.sum(axis=K) * d[c]                    # (32, 512, 512)
    ds  = avgpool4x4(xs)                          # (32, 128, 128)
    h1  = leaky_relu(ds.flat @ W1.T + b1, 0.2)    # (32, 1024)
    coef= leaky_relu(h1 @ W2.T + b2, 0.2)         # (32, 1)
    out = einsum('c,cwh->wh', coef, xs) / 32      # (512, 512)

Sharding: feature-parallel. Core m owns image rows [64m, 64m+64) of every
(b, c, k) plane — the same 64 MiB of x per core as batch-parallel, but W1
shards with the rows (each pool-row block only ever meets its own 2048
features of W1), so each core reads 4 MiB of W1 instead of 32 MiB.  The
h1 partial sums (feature-dim partials for all 256 (b, c) pairs) are
combined with a 512 KiB bf16 AllReduce; every core then computes the tiny
MLP tail redundantly and produces its own 64 output rows per sample.

Per-core dataflow, 32 tiles = (s4 strips of 16 rows) x (b8 samples):
  per tile (b, s), rows 64m+16s .. +16, all 32 channels, both k:
    - DMA xt [128p=(c,rb), (k2, r4, h512)] f32          (2 MiB, 8 KiB descs)
    - DVE: k-sum -> xs slab slice [128, 2048] bf16      (slab kept, 16 MiB)
    - DVE: 4x4-pool XY-reduce -> pool [128p=(c,rb), j128] bf16
    - PE : pool.T @ dmat -> ptps [j, (c,rb)] PSUM       (dmat = diag(d/16), bf16)
    - ACT: cast + reorder -> ptsb4 [j, rb, b, c] bf16   (gathers 4 samples)
  per (s, b-group of 4): h1p[(b',c), (bg,hid)] += ptsb4.T @ w1c  (M=128!)
  AllReduce h1 partials (bf16, 512 KiB), tiny MLP tail, build S32
  per tile: 4 matmuls M=32 (j-replicated) at PSUM quadrant offsets
  0/32/64/96 -> one full-width [128,512] PSUM->SBUF copy -> 4 row DMAs.
"""

import numpy as np
import ml_dtypes

B, C, K, W = 8, 32, 2, 512
P = 4                 # pool kernel/stride
G = W // P            # 128 pooled cols
FEAT = G * G          # 16384
HID = 1024
NEG = 0.2
N_CORES = 8
RPC = W // N_CORES    # 64 rows per core
S = 4                 # 16-row strips per sample per core
RB = 4                # pool row-groups per strip
T = B * S             # 32 tiles
SL = P * W            # slab slice free size per tile = 2048
FPC = FEAT // N_CORES # 2048 features per core

import os as _os
CC_MODE = _os.environ.get("BASS_CC_MODE", "ar")  # "ar" | "a2a"

_prog_cache = {}


def _build_program():
    import concourse.bass as bass
    import concourse.tile as tile
    from concourse import bacc, mybir

    dt = mybir.dt
    f32 = dt.float32
    bf16 = dt.bfloat16
    Alu = mybir.AluOpType
    AX = mybir.AxisListType

    nc = bacc.Bacc(
        "TRN2",
        debug=False,
        enable_asserts=False,
        target_bir_lowering=False,
        num_devices=N_CORES,
    )

    x_t = nc.dram_tensor("x", [B, C, K, RPC, W], f32, kind="ExternalInput").ap()
    w1s_t = nc.dram_tensor("w1s", [S, 128, RB, HID], bf16, kind="ExternalInput").ap()
    dmat_t = nc.dram_tensor("dmat", [128, 128], bf16, kind="ExternalInput").ap()
    b1r2_t = nc.dram_tensor("b1r2", [128, 2 * HID], bf16, kind="ExternalInput").ap()
    w2r2_t = nc.dram_tensor("w2r2", [128, 2 * HID], bf16, kind="ExternalInput").ap()
    b2r2_t = nc.dram_tensor("b2r2", [128, 2], f32, kind="ExternalInput").ap()
    mask4_t = nc.dram_tensor("mask4", [128, 4], f32, kind="ExternalInput").ap()
    tc128_t = nc.dram_tensor("tc128", [128, 128], f32, kind="ExternalInput").ap()
    m032_t = nc.dram_tensor("m032", [128, 32], bf16, kind="ExternalInput").ap()
    sum16_t = nc.dram_tensor("sum16", [128, 128], bf16, kind="ExternalInput").ap()
    out_t = nc.dram_tensor("out", [B, RPC, W], f32, kind="ExternalOutput").ap()

    with tile.TileContext(nc) as tc:
        with (
            tc.tile_pool(name="singles", bufs=1) as singles,
            tc.tile_pool(name="small", bufs=1) as small,
            tc.tile_pool(name="dram", bufs=4, space="DRAM") as dram,
        ):
            # constants
            dmat_sb = singles.tile([128, 128], bf16)
            nc.sync.dma_start(dmat_sb[:], dmat_t)
            b1r2_sb = singles.tile([128, 2 * HID], bf16)
            nc.sync.dma_start(b1r2_sb[:], b1r2_t)
            w2r2_sb = singles.tile([128, 2 * HID], bf16)
            nc.sync.dma_start(w2r2_sb[:], w2r2_t)
            b2r2_sb = singles.tile([128, 2], f32)
            nc.sync.dma_start(b2r2_sb[:], b2r2_t)
            mask4_sb = singles.tile([128, 4], f32)
            nc.sync.dma_start(mask4_sb[:], mask4_t)
            tc128_sb = singles.tile([128, 128], f32)
            nc.sync.dma_start(tc128_sb[:], tc128_t)
            m032_sb = singles.tile([128, 32], bf16)
            nc.sync.dma_start(m032_sb[:], m032_t)
            sum16_sb = singles.tile([128, 128], bf16)
            nc.sync.dma_start(sum16_sb[:], sum16_t)

            # xs slab: k-summed (unscaled) x, bf16,
            # [(c32 rb4), (b8 s4 r4 h512)]
            xs_sb = singles.tile([128, T * SL], bf16)
            h1b_sb = singles.tile([128, 2 * HID], bf16)   # pre-AR partial
            h1r_sb = singles.tile([128, 2 * HID], bf16)   # post-AR sum
            S32_sb = singles.tile([128, B, 32], bf16)

            arin = dram.tile([128, 2 * HID], bf16)
            arout = dram.tile([128, 2 * HID], bf16)
            agin = dram.tile([16, 2], f32)
            agout = dram.tile([128, 2], f32)

            with tc.tile_pool(name="psum_h1", bufs=1, space="PSUM") as ph1:
                h1p = ph1.tile([128, 2 * HID], f32)

                with (
                    tc.tile_pool(name="xt", bufs=2) as xtp,
                    tc.tile_pool(name="w1c", bufs=2) as w1p,
                    tc.tile_pool(name="pool", bufs=2) as poolp,
                    tc.tile_pool(name="poolb", bufs=2) as poolbp,
                    tc.tile_pool(name="ptsb", bufs=2) as ptp,
                    tc.tile_pool(name="psum_pt", bufs=2, space="PSUM") as pptp,
                ):
                    for s in range(S):
                        w1c = w1p.tile([128, RB, HID], bf16)
                        nc.sync.dma_start(w1c[:], w1s_t[s])
                        ptsb4 = ptp.tile([128, RB, B, C], bf16)

                        for b in range(B):
                            xt = xtp.tile([128, K, P * W], f32)
                            # descriptors: 4 contiguous image rows (8 KiB),
                            # outermost dim c=32 spreads over SDMA engines.
                            for k in range(K):
                                xg = x_t[b, :, k, 16 * s : 16 * (s + 1), :].rearrange(
                                    "c (rb r) h -> c rb (r h)", rb=RB
                                )
                                nc.sync.dma_start(xt[:, k, :], xg)

                            t = S * b + s
                            xs_slice = xs_sb[:, SL * t : SL * (t + 1)]
                            nc.vector.tensor_add(xs_slice, xt[:, 0, :], xt[:, 1, :])

                            # 4x4 avgpool (sum; /16 and d_c fold into dmat)
                            pool = poolp.tile([128, G], f32)
                            nc.vector.reduce_sum(
                                pool[:],
                                xs_slice.rearrange("p (r j f) -> p j r f", r=P, f=P),
                                axis=AX.XY,
                            )
                            # cast on ACT so the transpose matmul runs bf16
                            poolb = poolbp.tile([128, G], bf16)
                            nc.scalar.copy(poolb[:], pool[:])

                            # transpose + scale: ptps[j, (c,rb)] = pool[(c,rb), j]*d_c/16
                            ptps = pptp.tile([128, 128], f32)
                            nc.tensor.matmul(
                                ptps[:], lhsT=poolb[:], rhs=dmat_sb[:],
                                start=True, stop=True,
                            )
                            # cast bf16; reorder (c,rb)->(rb,c); scatter to b col
                            nc.scalar.copy(
                                ptsb4[:, :, b, :],
                                ptps[:].rearrange("j (c rb) -> j rb c", rb=RB),
                            )

                            if b % 4 == 3:
                                bg = b // 4
                                for rb in range(RB):
                                    lhs = ptsb4[:, rb, 4 * bg : 4 * bg + 4, :]
                                    first = s == 0 and rb == 0
                                    last = s == S - 1 and rb == RB - 1
                                    for h in range(2):
                                        nc.tensor.matmul(
                                            h1p[:, HID * bg + 512 * h :
                                                   HID * bg + 512 * (h + 1)],
                                            lhsT=lhs,
                                            rhs=w1c[:, rb, 512 * h : 512 * (h + 1)],
                                            start=first, stop=last,
                                        )

                # cast h1 partial to bf16, folding in b1/8 (so the AR sum
                # carries the bias exactly once)
                nc.vector.tensor_add(h1b_sb[:], h1p[:], b1r2_sb[:])

            u2 = small.tile([128, 2], f32)
            if CC_MODE == "ar":
                # ---- AllReduce of h1 partials across the 8 cores ----
                # (single cc: splitting into two pipelined cc's measured
                # slower; overlapping a cc with the phase-1 x-stream runs it
                # 2x slower)
                nc.gpsimd.dma_start(arin[:], h1b_sb[:])
                nc.gpsimd.collective_compute(
                    "AllReduce", Alu.add,
                    replica_groups=[list(range(N_CORES))],
                    ins=[arin.opt()], outs=[arout.opt()],
                )
                # return DMA on sync: keeps the gpsimd DRAIN off the
                # critical path
                nc.sync.dma_start(h1r_sb[:], arout[:])

                # ---- MLP tail (per (b', c) partition, bg in free dim) ----
                nc.vector.scalar_tensor_tensor(
                    out=h1b_sb[:], in0=h1r_sb[:], scalar=NEG, in1=h1r_sb[:],
                    op0=Alu.mult, op1=Alu.max,
                )
                nc.vector.tensor_mul(h1r_sb[:], h1b_sb[:], w2r2_sb[:])
                red = small.tile([128, 2], f32)
                nc.vector.reduce_sum(
                    red[:],
                    h1r_sb[:].rearrange("p (bg hid) -> p bg hid", bg=2),
                    axis=AX.X,
                )
                cf = small.tile([128, 2], f32)
                nc.vector.tensor_add(cf[:], red[:], b2r2_sb[:])
                nc.vector.scalar_tensor_tensor(
                    out=u2[:], in0=cf[:], scalar=NEG, in1=cf[:],
                    op0=Alu.mult, op1=Alu.max,
                )
            else:
                # ---- AllToAll: core m receives every core's partial for h1
                # rows 16m..16m+16; local PE sum; tiny tail; AllGather u ----
                nc.gpsimd.dma_start(arin[:], h1b_sb[:])
                nc.gpsimd.collective_compute(
                    "AllToAll", Alu.bypass,
                    replica_groups=[list(range(N_CORES))],
                    ins=[arin.opt()], outs=[arout.opt()],
                )
                nc.sync.dma_start(h1r_sb[:], arout[:])
                with tc.tile_pool(name="psum_a2a", bufs=1, space="PSUM") as pa:
                    # M=128 replicated sum (8 copies of the 16 rows): walrus
                    # rejects sub-32-partition PSUM consumers, so stay
                    # full-width; the redundancy is free.
                    h1srep = pa.tile([128, 2 * HID], f32)
                    for ch in range(4):
                        nc.tensor.matmul(
                            h1srep[:, 512 * ch : 512 * (ch + 1)],
                            lhsT=sum16_sb[:],
                            rhs=h1r_sb[:, 512 * ch : 512 * (ch + 1)],
                            start=True, stop=True,
                        )
                    # STT can't take PSUM inputs (walrus InstTensorScalarPtr
                    # verifier) — copy to SBUF first, halves on DVE + ACT
                    nc.vector.tensor_copy(h1r_sb[:, 0:HID], h1srep[:, 0:HID])
                    nc.scalar.copy(h1r_sb[:, HID:], h1srep[:, HID:])
                nc.vector.scalar_tensor_tensor(
                    out=h1b_sb[:], in0=h1r_sb[:], scalar=NEG,
                    in1=h1r_sb[:], op0=Alu.mult, op1=Alu.max,
                )
                nc.vector.tensor_mul(h1r_sb[:], h1b_sb[:], w2r2_sb[:])
                red16 = small.tile([128, 2], f32)
                nc.vector.reduce_sum(
                    red16[:],
                    h1r_sb[:].rearrange("p (bg hid) -> p bg hid", bg=2),
                    axis=AX.X,
                )
                cf16 = small.tile([128, 2], f32)
                nc.vector.tensor_add(cf16[:], red16[:], b2r2_sb[:])
                u16 = small.tile([128, 2], f32)
                nc.vector.scalar_tensor_tensor(
                    out=u16[:], in0=cf16[:], scalar=NEG, in1=cf16[:],
                    op0=Alu.mult, op1=Alu.max,
                )
                nc.gpsimd.dma_start(agin[:], u16[0:16, :])
                nc.gpsimd.collective_compute(
                    "AllGather", Alu.bypass,
                    replica_groups=[list(range(N_CORES))],
                    ins=[agin.opt()], outs=[agout.opt()],
                )
                nc.sync.dma_start(u2[:], agout[:])

            # tmp8[q=(b',c), b] = coef[q, bg]*delta(b', b%4)
            tmp8 = small.tile([128, 8], f32)
            nc.vector.tensor_scalar_mul(tmp8[:, 0:4], mask4_sb[:], u2[:, 0:1])
            nc.vector.tensor_scalar_mul(tmp8[:, 4:8], mask4_sb[:], u2[:, 1:2])

            with (
                tc.tile_pool(name="psum_s8", bufs=1, space="PSUM") as ps8,
                tc.tile_pool(name="psum_o", bufs=6, space="PSUM") as pop,
                tc.tile_pool(name="ob", bufs=6) as obp,
            ):
                s8 = ps8.tile([128, 8], f32)
                # S8[p=(c,rb), b] = coef[b, c] * d_c / 32
                nc.tensor.matmul(
                    s8[:], lhsT=tc128_sb[:], rhs=tmp8[:], start=True, stop=True,
                )
                # S32[p, b, (rb'4, j8)] = delta(rb(p), rb') * u_{b, c(p)}
                for b in range(B):
                    nc.vector.tensor_scalar_mul(
                        S32_sb[:, b, :], m032_sb[:], s8[:, b : b + 1]
                    )

                # ---- weighted channel sum ----
                # Per tile: 4 matmuls (M=32, j-replicated) at PSUM partition
                # offsets 0/32/64/96, one full-width PSUM->SBUF copy, one
                # strided-partition DMA.
                for t in range(T):
                    b, s = t // S, t % S
                    po = pop.tile([128, 512], f32)
                    for r in range(P):
                        nc.tensor.matmul(
                            po[32 * r : 32 * (r + 1), :],
                            lhsT=S32_sb[:, b, :],
                            rhs=xs_sb[:, SL * t + 512 * r : SL * t + 512 * (r + 1)],
                            start=True, stop=True,
                            tile_position=(0, 32 * r),
                        )
                    ob = obp.tile([128, 512], f32)
                    if t % 2 == 0:
                        nc.vector.tensor_copy(ob[:], po[:])
                    else:
                        nc.scalar.copy(ob[:], po[:])
                    # useful rows (j=0 replicas) on partitions 8*(4r+rb),
                    # ordered (r, rb) -> strip row 4*rb + r; single DMA
                    og = out_t[b, 16 * s : 16 * (s + 1), :].rearrange(
                        "(rb r) h -> r rb h", rb=RB
                    )
                    src = ob[:].rearrange("(q e) h -> q e h", e=8)[:, 0, :]
                    nc.sync.dma_start(og, src)

    nc.compile()
    return nc


def _get_program():
    if "nc" not in _prog_cache:
        _prog_cache["nc"] = _build_program()
    return _prog_cache["nc"]


def prep_in_maps(x, dist, W1, b1, W2, b2):
    bf16 = ml_dtypes.bfloat16
    x = np.asarray(x, dtype=np.float32)
    dist = np.asarray(dist, dtype=np.float32)
    W1 = np.asarray(W1, dtype=np.float32)
    b1 = np.asarray(b1, dtype=np.float32)
    W2 = np.asarray(W2, dtype=np.float32)
    b2 = np.asarray(b2, dtype=np.float32)

    d = (dist / dist.sum()).astype(np.float32)
    dr = np.repeat(d, RB)  # d[p//4] for p=(c,rb)

    # dmat[p, q] = delta(p, q) * d[q//4] / 16
    dmat = np.diag(dr / (P * P)).astype(bf16)
    # b1 tiled over bg, pre-divided by N_CORES (folded pre-AllReduce)
    b1r2 = np.broadcast_to(np.tile(b1, 2) / N_CORES, (128, 2 * HID)).astype(bf16)
    b1r2 = np.ascontiguousarray(b1r2)
    w2r2 = np.ascontiguousarray(
        np.broadcast_to(np.tile(W2[0], 2), (128, 2 * HID))
    ).astype(bf16)
    b2r2 = np.full((128, 2), b2[0], dtype=np.float32)
    # mask4[p=(b',c), j] = delta(b', j)
    pp = np.arange(128)
    mask4 = (pp[:, None] // 32 == np.arange(4)[None, :]).astype(np.float32)
    # tc128[q=(b',c'), p=(c,rb)] = delta(c', c) * d_c / 32
    tc128 = ((pp[:, None] % 32) == (pp[None, :] // 4)).astype(np.float32)
    tc128 *= dr[None, :] / C
    # m032[p=(c,rb), m=(rb'4, j8)] = delta(rb, rb')
    m032 = (pp[:, None] % 4 == np.arange(32)[None, :] // 8).astype(bf16)
    # sum16[p=(i8,q16), m] = delta(q, m%16): sums the 8 A2A blocks, M=128
    # replicated so downstream ops stay full-width
    sum16 = (pp[:, None] % 16 == np.arange(128)[None, :] % 16).astype(bf16)

    # W1 slices: core m needs features [2048m, 2048(m+1)) of W1T
    w1t = np.ascontiguousarray(W1.T).astype(bf16)  # [FEAT, HID]

    in_maps = []
    for m in range(N_CORES):
        x_own = np.ascontiguousarray(x[:, :, :, RPC * m : RPC * (m + 1), :])
        w1s = np.ascontiguousarray(
            w1t[FPC * m : FPC * (m + 1)]
            .reshape(S, RB, 128, HID)
            .transpose(0, 2, 1, 3)
        )
        in_maps.append(
            dict(
                x=x_own,
                w1s=w1s,
                dmat=dmat,
                b1r2=b1r2,
                w2r2=w2r2,
                b2r2=b2r2,
                mask4=mask4,
                tc128=tc128,
                m032=m032,
                sum16=sum16,
            )
        )
    return in_maps


def kernel(x, dist, W1, b1, W2, b2):
    from concourse.bass_utils import run_bass_kernel_spmd

    in_maps = prep_in_maps(x, dist, W1, b1, W2, b2)
    nc = _get_program()
    res = run_bass_kernel_spmd(nc, in_maps, list(range(N_CORES)))
    out = np.empty((B, 1, W, W), dtype=np.float32)
    for m in range(N_CORES):
        out[:, 0, RPC * m : RPC * (m + 1), :] = res.results[m]["out"]
    return out


# revision 32
# speedup vs baseline: 1.0176x; 1.0176x over previous
"""Trainium2 Bass kernel for nn_MixChan (dense_mlp).

Reference computation (per batch sample b):
    d   = dist / dist.sum()                       # (32,)
    xs  = x.sum(axis=K) * d[c]                    # (32, 512, 512)
    ds  = avgpool4x4(xs)                          # (32, 128, 128)
    h1  = leaky_relu(ds.flat @ W1.T + b1, 0.2)    # (32, 1024)
    coef= leaky_relu(h1 @ W2.T + b2, 0.2)         # (32, 1)
    out = einsum('c,cwh->wh', coef, xs) / 32      # (512, 512)

Sharding: feature-parallel. Core m owns image rows [64m, 64m+64) of every
(b, c, k) plane — the same 64 MiB of x per core as batch-parallel, but W1
shards with the rows (each pool-row block only ever meets its own 2048
features of W1), so each core reads 4 MiB of W1 instead of 32 MiB.  The
h1 partial sums (feature-dim partials for all 256 (b, c) pairs) are
combined with a 512 KiB bf16 AllReduce; every core then computes the tiny
MLP tail redundantly and produces its own 64 output rows per sample.

Per-core dataflow, 32 tiles = (s4 strips of 16 rows) x (b8 samples):
  per tile (b, s), rows 64m+16s .. +16, all 32 channels, both k:
    - DMA xt [128p=(c,rb), (k2, r4, h512)] f32          (2 MiB, 8 KiB descs)
    - DVE: k-sum -> xs slab slice [128, 2048] bf16      (slab kept, 16 MiB)
    - DVE: 4x4-pool XY-reduce -> pool [128p=(c,rb), j128] bf16
    - PE : pool.T @ dmat -> ptps [j, (c,rb)] PSUM       (dmat = diag(d/16), bf16)
    - ACT: cast + reorder -> ptsb4 [j, rb, b, c] bf16   (gathers 4 samples)
  per (s, b-group of 4): h1p[(b',c), (bg,hid)] += ptsb4.T @ w1c  (M=128!)
  AllReduce h1 partials (bf16, 512 KiB), tiny MLP tail, build S32
  per tile: 4 matmuls M=32 (j-replicated) at PSUM quadrant offsets
  0/32/64/96 -> one full-width [128,512] PSUM->SBUF copy -> 4 row DMAs.
"""

import numpy as np
import ml_dtypes

B, C, K, W = 8, 32, 2, 512
P = 4                 # pool kernel/stride
G = W // P            # 128 pooled cols
FEAT = G * G          # 16384
HID = 1024
NEG = 0.2
N_CORES = 8
RPC = W // N_CORES    # 64 rows per core
S = 4                 # 16-row strips per sample per core
RB = 4                # pool row-groups per strip
T = B * S             # 32 tiles
SL = P * W            # slab slice free size per tile = 2048
FPC = FEAT // N_CORES # 2048 features per core

import os as _os
CC_MODE = _os.environ.get("BASS_CC_MODE", "ar")  # "ar" | "a2a"
LRELU_ACT = _os.environ.get("BASS_LRELU_ACT", "0") == "1"  # ACT Lrelu ignores alpha on this stack -> wrong results
STT_ACCUM = _os.environ.get("BASS_STT_ACCUM", "1") == "1"

_prog_cache = {}


def _build_program():
    import concourse.bass as bass
    import concourse.tile as tile
    from concourse import bacc, mybir

    dt = mybir.dt
    f32 = dt.float32
    bf16 = dt.bfloat16
    Alu = mybir.AluOpType
    AX = mybir.AxisListType

    nc = bacc.Bacc(
        "TRN2",
        debug=False,
        enable_asserts=False,
        target_bir_lowering=False,
        num_devices=N_CORES,
    )

    x_t = nc.dram_tensor("x", [B, C, K, RPC, W], f32, kind="ExternalInput").ap()
    w1s_t = nc.dram_tensor("w1s", [S, 128, RB, HID], bf16, kind="ExternalInput").ap()
    dmat_t = nc.dram_tensor("dmat", [128, 128], bf16, kind="ExternalInput").ap()
    b1r2_t = nc.dram_tensor("b1r2", [128, 2 * HID], bf16, kind="ExternalInput").ap()
    w2r2_t = nc.dram_tensor("w2r2", [128, 2 * HID], bf16, kind="ExternalInput").ap()
    b2r2_t = nc.dram_tensor("b2r2", [128, 2], f32, kind="ExternalInput").ap()
    mask4_t = nc.dram_tensor("mask4", [128, 4], f32, kind="ExternalInput").ap()
    tc128_t = nc.dram_tensor("tc128", [128, 128], f32, kind="ExternalInput").ap()
    m032_t = nc.dram_tensor("m032", [128, 32], bf16, kind="ExternalInput").ap()
    sum16_t = nc.dram_tensor("sum16", [128, 128], bf16, kind="ExternalInput").ap()
    out_t = nc.dram_tensor("out", [B, RPC, W], f32, kind="ExternalOutput").ap()

    with tile.TileContext(nc) as tc:
        with (
            tc.tile_pool(name="singles", bufs=1) as singles,
            tc.tile_pool(name="small", bufs=1) as small,
            tc.tile_pool(name="dram", bufs=4, space="DRAM") as dram,
        ):
            # constants
            dmat_sb = singles.tile([128, 128], bf16)
            nc.sync.dma_start(dmat_sb[:], dmat_t)
            b1r2_sb = singles.tile([128, 2 * HID], bf16)
            nc.sync.dma_start(b1r2_sb[:], b1r2_t)
            w2r2_sb = singles.tile([128, 2 * HID], bf16)
            nc.sync.dma_start(w2r2_sb[:], w2r2_t)
            b2r2_sb = singles.tile([128, 2], f32)
            nc.sync.dma_start(b2r2_sb[:], b2r2_t)
            mask4_sb = singles.tile([128, 4], f32)
            nc.sync.dma_start(mask4_sb[:], mask4_t)
            tc128_sb = singles.tile([128, 128], f32)
            nc.sync.dma_start(tc128_sb[:], tc128_t)
            m032_sb = singles.tile([128, 32], bf16)
            nc.sync.dma_start(m032_sb[:], m032_t)
            sum16_sb = singles.tile([128, 128], bf16)
            nc.sync.dma_start(sum16_sb[:], sum16_t)

            # xs slab: k-summed (unscaled) x, bf16,
            # [(c32 rb4), (b8 s4 r4 h512)]
            xs_sb = singles.tile([128, T * SL], bf16)
            h1b_sb = singles.tile([128, 2 * HID], bf16)   # pre-AR partial
            h1r_sb = singles.tile([128, 2 * HID], bf16)   # post-AR sum
            S32_sb = singles.tile([128, B, 32], bf16)

            arin = dram.tile([128, 2 * HID], bf16)
            arout = dram.tile([128, 2 * HID], bf16)
            agin = dram.tile([16, 2], f32)
            agout = dram.tile([128, 2], f32)

            with tc.tile_pool(name="psum_h1", bufs=1, space="PSUM") as ph1:
                h1p = ph1.tile([128, 2 * HID], f32)

                with (
                    tc.tile_pool(name="xt", bufs=2) as xtp,
                    tc.tile_pool(name="w1c", bufs=2) as w1p,
                    tc.tile_pool(name="pool", bufs=2) as poolp,
                    tc.tile_pool(name="poolb", bufs=2) as poolbp,
                    tc.tile_pool(name="ptsb", bufs=2) as ptp,
                    tc.tile_pool(name="psum_pt", bufs=2, space="PSUM") as pptp,
                ):
                    for s in range(S):
                        w1c = w1p.tile([128, RB, HID], bf16)
                        nc.sync.dma_start(w1c[:], w1s_t[s])
                        ptsb4 = ptp.tile([128, RB, B, C], bf16)

                        for b in range(B):
                            xt = xtp.tile([128, K, P * W], f32)
                            # descriptors: 4 contiguous image rows (8 KiB),
                            # outermost dim c=32 spreads over SDMA engines.
                            for k in range(K):
                                xg = x_t[b, :, k, 16 * s : 16 * (s + 1), :].rearrange(
                                    "c (rb r) h -> c rb (r h)", rb=RB
                                )
                                nc.sync.dma_start(xt[:, k, :], xg)

                            t = S * b + s
                            xs_slice = xs_sb[:, SL * t : SL * (t + 1)]
                            nc.vector.tensor_add(xs_slice, xt[:, 0, :], xt[:, 1, :])

                            # 4x4 avgpool (sum; /16 and d_c fold into dmat)
                            pool = poolp.tile([128, G], f32)
                            nc.vector.reduce_sum(
                                pool[:],
                                xs_slice.rearrange("p (r j f) -> p j r f", r=P, f=P),
                                axis=AX.XY,
                            )
                            # cast on ACT so the transpose matmul runs bf16
                            poolb = poolbp.tile([128, G], bf16)
                            nc.scalar.copy(poolb[:], pool[:])

                            # transpose + scale: ptps[j, (c,rb)] = pool[(c,rb), j]*d_c/16
                            ptps = pptp.tile([128, 128], f32)
                            nc.tensor.matmul(
                                ptps[:], lhsT=poolb[:], rhs=dmat_sb[:],
                                start=True, stop=True,
                            )
                            # cast bf16; reorder (c,rb)->(rb,c); scatter to b col
                            nc.scalar.copy(
                                ptsb4[:, :, b, :],
                                ptps[:].rearrange("j (c rb) -> j rb c", rb=RB),
                            )

                            if b % 4 == 3:
                                bg = b // 4
                                for rb in range(RB):
                                    lhs = ptsb4[:, rb, 4 * bg : 4 * bg + 4, :]
                                    first = s == 0 and rb == 0
                                    last = s == S - 1 and rb == RB - 1
                                    for h in range(2):
                                        nc.tensor.matmul(
                                            h1p[:, HID * bg + 512 * h :
                                                   HID * bg + 512 * (h + 1)],
                                            lhsT=lhs,
                                            rhs=w1c[:, rb, 512 * h : 512 * (h + 1)],
                                            start=first, stop=last,
                                        )

                # cast h1 partial to bf16, folding in b1/8 (so the AR sum
                # carries the bias exactly once)
                nc.vector.tensor_add(h1b_sb[:], h1p[:], b1r2_sb[:])

            u2 = small.tile([128, 2], f32)
            red = small.tile([128, 2], f32)
            cf = small.tile([128, 2], f32)
            Act = mybir.ActivationFunctionType
            if CC_MODE == "ar":
                # ---- AllReduce of h1 partials across the 8 cores ----
                # (single cc: splitting into two pipelined cc's measured
                # slower; overlapping a cc with the phase-1 x-stream runs it
                # 2x slower)
                nc.sync.dma_start(arin[:], h1b_sb[:])
                nc.gpsimd.collective_compute(
                    "AllReduce", Alu.add,
                    replica_groups=[list(range(N_CORES))],
                    ins=[arin.opt()], outs=[arout.opt()],
                )
                # return DMA on sync: keeps the gpsimd DRAIN off the
                # critical path
                nc.sync.dma_start(h1r_sb[:], arout[:])
                # leaky on ACT
                if LRELU_ACT:
                    nc.scalar.activation(
                        h1b_sb[:], h1r_sb[:], Act.Lrelu, alpha=NEG
                    )
                else:
                    nc.vector.scalar_tensor_tensor(
                        out=h1b_sb[:], in0=h1r_sb[:], scalar=NEG,
                        in1=h1r_sb[:], op0=Alu.mult, op1=Alu.max,
                    )
            else:
                # ---- AllToAll: core m receives every core's partial for h1
                # rows 16m..16m+16; local PE sum (M=128 replicated: walrus
                # rejects sub-32-partition PSUM consumers); tail; AllGather u
                nc.sync.dma_start(arin[:], h1b_sb[:])
                nc.gpsimd.collective_compute(
                    "AllToAll", Alu.bypass,
                    replica_groups=[list(range(N_CORES))],
                    ins=[arin.opt()], outs=[arout.opt()],
                )
                nc.sync.dma_start(h1r_sb[:], arout[:])
                with tc.tile_pool(name="psum_a2a", bufs=1, space="PSUM") as pa:
                    h1srep = pa.tile([128, 2 * HID], f32)
                    for ch in range(4):
                        nc.tensor.matmul(
                            h1srep[:, 512 * ch : 512 * (ch + 1)],
                            lhsT=sum16_sb[:],
                            rhs=h1r_sb[:, 512 * ch : 512 * (ch + 1)],
                            start=True, stop=True,
                        )
                    # leaky on ACT, straight from PSUM
                    if LRELU_ACT:
                        nc.scalar.activation(
                            h1b_sb[:], h1srep[:], Act.Lrelu, alpha=NEG
                        )
                    else:
                        nc.vector.tensor_copy(
                            h1r_sb[:, 0:HID], h1srep[:, 0:HID]
                        )
                        nc.scalar.copy(h1r_sb[:, HID:], h1srep[:, HID:])
                if not LRELU_ACT:
                    nc.vector.scalar_tensor_tensor(
                        out=h1b_sb[:], in0=h1r_sb[:], scalar=NEG,
                        in1=h1r_sb[:], op0=Alu.mult, op1=Alu.max,
                    )

            # W2 dot: (h1s * 1.0) * w2, with per-bg free-dim accumulator
            if STT_ACCUM:
                for bg in range(2):
                    hs = slice(HID * bg, HID * (bg + 1))
                    nc.vector.scalar_tensor_tensor(
                        out=h1r_sb[:, hs], in0=h1b_sb[:, hs], scalar=1.0,
                        in1=w2r2_sb[:, hs], op0=Alu.mult, op1=Alu.mult,
                        accum_out=red[:, bg : bg + 1],
                    )
            else:
                nc.vector.tensor_mul(h1r_sb[:], h1b_sb[:], w2r2_sb[:])
                nc.vector.reduce_sum(
                    red[:],
                    h1r_sb[:].rearrange("p (bg hid) -> p bg hid", bg=2),
                    axis=AX.X,
                )
            nc.vector.tensor_add(cf[:], red[:], b2r2_sb[:])
            if CC_MODE == "ar":
                nc.vector.scalar_tensor_tensor(
                    out=u2[:], in0=cf[:], scalar=NEG, in1=cf[:],
                    op0=Alu.mult, op1=Alu.max,
                )
            else:
                u16 = small.tile([128, 2], f32)
                nc.vector.scalar_tensor_tensor(
                    out=u16[:], in0=cf[:], scalar=NEG, in1=cf[:],
                    op0=Alu.mult, op1=Alu.max,
                )
                nc.gpsimd.dma_start(agin[:], u16[0:16, :])
                nc.gpsimd.collective_compute(
                    "AllGather", Alu.bypass,
                    replica_groups=[list(range(N_CORES))],
                    ins=[agin.opt()], outs=[agout.opt()],
                )
                nc.sync.dma_start(u2[:], agout[:])

            # tmp8[q=(b',c), b] = coef[q, bg]*delta(b', b%4)
            tmp8 = small.tile([128, 8], f32)
            nc.vector.tensor_scalar_mul(tmp8[:, 0:4], mask4_sb[:], u2[:, 0:1])
            nc.vector.tensor_scalar_mul(tmp8[:, 4:8], mask4_sb[:], u2[:, 1:2])

            with (
                tc.tile_pool(name="psum_s8", bufs=1, space="PSUM") as ps8,
                tc.tile_pool(name="psum_o", bufs=6, space="PSUM") as pop,
                tc.tile_pool(name="ob", bufs=6) as obp,
            ):
                s8 = ps8.tile([128, 8], f32)
                # S8[p=(c,rb), b] = coef[b, c] * d_c / 32
                nc.tensor.matmul(
                    s8[:], lhsT=tc128_sb[:], rhs=tmp8[:], start=True, stop=True,
                )
                # S32[p, b, (rb'4, j8)] = delta(rb(p), rb') * u_{b, c(p)}
                for b in range(B):
                    nc.vector.tensor_scalar_mul(
                        S32_sb[:, b, :], m032_sb[:], s8[:, b : b + 1]
                    )

                # ---- weighted channel sum ----
                # Per tile: 4 matmuls (M=32, j-replicated) at PSUM partition
                # offsets 0/32/64/96, one full-width PSUM->SBUF copy, one
                # strided-partition DMA.
                for t in range(T):
                    b, s = t // S, t % S
                    po = pop.tile([128, 512], f32)
                    for r in range(P):
                        nc.tensor.matmul(
                            po[32 * r : 32 * (r + 1), :],
                            lhsT=S32_sb[:, b, :],
                            rhs=xs_sb[:, SL * t + 512 * r : SL * t + 512 * (r + 1)],
                            start=True, stop=True,
                            tile_position=(0, 32 * r),
                        )
                    ob = obp.tile([128, 512], f32)
                    if t % 2 == 0:
                        nc.vector.tensor_copy(ob[:], po[:])
                    else:
                        nc.scalar.copy(ob[:], po[:])
                    # useful rows (j=0 replicas) on partitions 8*(4r+rb),
                    # ordered (r, rb) -> strip row 4*rb + r; single DMA
                    og = out_t[b, 16 * s : 16 * (s + 1), :].rearrange(
                        "(rb r) h -> r rb h", rb=RB
                    )
                    src = ob[:].rearrange("(q e) h -> q e h", e=8)[:, 0, :]
                    nc.sync.dma_start(og, src)

    nc.compile()
    return nc


def _get_program():
    if "nc" not in _prog_cache:
        _prog_cache["nc"] = _build_program()
    return _prog_cache["nc"]


def prep_in_maps(x, dist, W1, b1, W2, b2):
    bf16 = ml_dtypes.bfloat16
    x = np.asarray(x, dtype=np.float32)
    dist = np.asarray(dist, dtype=np.float32)
    W1 = np.asarray(W1, dtype=np.float32)
    b1 = np.asarray(b1, dtype=np.float32)
    W2 = np.asarray(W2, dtype=np.float32)
    b2 = np.asarray(b2, dtype=np.float32)

    d = (dist / dist.sum()).astype(np.float32)
    dr = np.repeat(d, RB)  # d[p//4] for p=(c,rb)

    # dmat[p, q] = delta(p, q) * d[q//4] / 16
    dmat = np.diag(dr / (P * P)).astype(bf16)
    # b1 tiled over bg, pre-divided by N_CORES (folded pre-AllReduce)
    b1r2 = np.broadcast_to(np.tile(b1, 2) / N_CORES, (128, 2 * HID)).astype(bf16)
    b1r2 = np.ascontiguousarray(b1r2)
    w2r2 = np.ascontiguousarray(
        np.broadcast_to(np.tile(W2[0], 2), (128, 2 * HID))
    ).astype(bf16)
    b2r2 = np.full((128, 2), b2[0], dtype=np.float32)
    # mask4[p=(b',c), j] = delta(b', j)
    pp = np.arange(128)
    mask4 = (pp[:, None] // 32 == np.arange(4)[None, :]).astype(np.float32)
    # tc128[q=(b',c'), p=(c,rb)] = delta(c', c) * d_c / 32
    tc128 = ((pp[:, None] % 32) == (pp[None, :] // 4)).astype(np.float32)
    tc128 *= dr[None, :] / C
    # m032[p=(c,rb), m=(rb'4, j8)] = delta(rb, rb')
    m032 = (pp[:, None] % 4 == np.arange(32)[None, :] // 8).astype(bf16)
    # sum16[p=(i8,q16), m] = delta(q, m%16): sums the 8 A2A blocks, M=128
    # replicated so downstream ops stay full-width
    sum16 = (pp[:, None] % 16 == np.arange(128)[None, :] % 16).astype(bf16)

    # W1 slices: core m needs features [2048m, 2048(m+1)) of W1T
    w1t = np.ascontiguousarray(W1.T).astype(bf16)  # [FEAT, HID]

    in_maps = []
    for m in range(N_CORES):
        x_own = np.ascontiguousarray(x[:, :, :, RPC * m : RPC * (m + 1), :])
        w1s = np.ascontiguousarray(
            w1t[FPC * m : FPC * (m + 1)]
            .reshape(S, RB, 128, HID)
            .transpose(0, 2, 1, 3)
        )
        in_maps.append(
            dict(
                x=x_own,
                w1s=w1s,
                dmat=dmat,
                b1r2=b1r2,
                w2r2=w2r2,
                b2r2=b2r2,
                mask4=mask4,
                tc128=tc128,
                m032=m032,
                sum16=sum16,
            )
        )
    return in_maps


def kernel(x, dist, W1, b1, W2, b2):
    from concourse.bass_utils import run_bass_kernel_spmd

    in_maps = prep_in_maps(x, dist, W1, b1, W2, b2)
    nc = _get_program()
    res = run_bass_kernel_spmd(nc, in_maps, list(range(N_CORES)))
    out = np.empty((B, 1, W, W), dtype=np.float32)
    for m in range(N_CORES):
        out[:, 0, RPC * m : RPC * (m + 1), :] = res.results[m]["out"]
    return out
